# revision 16
# baseline (speedup 1.0000x reference)
"""Trainium2 Bass kernel for nn_AttentionRNNCell (B=4096, D=H=A=1024, F=4096).

Math (after structural simplification of the reference):
  1) softmax over the size-1 k_len axis is identically 1.0, so
     attn = kx = inputs @ w_k.T + b_k, and qx / mlp_w / tanh are dead code.
  2) attn_out is then a LINEAR map of inputs, so the first FFN layer folds
     over it exactly (host-side fp32 precompute):
        ffn_w1 = [W1x | W1a]   (split along the input dim: D then A)
        W_eff  = W1x + W1a @ (w_proj @ w_k)            [F, D]
        b_eff  = ffn_b1 + W1a @ (w_proj @ b_k + b_proj)
  Device computation per batch row:
    h1      = relu(x @ W_eff.T + b_eff)          [B, F]   (K=1024)
    ffn_out = h1 @ ffn_w2.T + ffn_b2             [B, D+A] (K=4096)
    new_h   = cat(x, hidden) @ w_out.T + b_out   [B, H]   (K=2048)
    output  = log_softmax(ffn_out, axis=-1)
  returns (output, new_h)

Sharding: data-parallel over B across 8 cores (512 rows/core), weights
replicated.  The log_softmax branch (C/D) runs bf16 matmuls with fp32 PSUM
accumulation -- log_softmax normalizes away most quantization noise.  The
new_hidden GEMM (E) runs fp32r (FP22 multiply, fp32 accumulate) since that
output has no normalizer.  Activations are feature-major ([feat, batch]) in
C; D and E put batch on partitions (activations stationary) so log_softmax
reduces along the free axis.  Biases fuse into the PSUM->SBUF activation
copy (C) or enter PSUM via K=1 ones-matmuls (D/E).  Weight streams alternate
between the HWDGE (sync) and SWDGE (gpsimd) DMA rings.
"""

import numpy as np
import ml_dtypes

import concourse.bass as bass
import concourse.mybir as mybir
import concourse.tile as tile
from concourse import bacc
from concourse.bass_utils import run_bass_kernel_spmd

F32 = mybir.dt.float32
F32R = mybir.dt.float32r
BF16 = mybir.dt.bfloat16
AFT = mybir.ActivationFunctionType
ALU = mybir.AluOpType
AX = mybir.AxisListType

NCORES = 8
B, D, H, A, F = 4096, 1024, 1024, 1024, 4096
C2 = D + A  # 2048, ffn output width and k-dim of w_out
BS = B // NCORES  # 512 batch rows per core

DT = D // 128  # 8   d-tiles (K of the fused first GEMM)
HT = H // 128  # 8   h-tiles
CT = C2 // 128  # 16  c-tiles (K of the w_out GEMM)
FT = F // 128  # 32  f-tiles (K of the second FFN GEMM)
MT = BS // 128  # 4   batch tiles per core


def _build_nc():
    nc = bacc.Bacc("TRN2", target_bir_lowering=False, debug=False)

    # ---- DRAM I/O (per-core shapes; host pre-tiles everything) ----
    xTb = nc.dram_tensor("xTb", [DT, 128, BS], BF16, kind="ExternalInput").ap()
    xT = nc.dram_tensor("xT", [DT, 128, BS], F32R, kind="ExternalInput").ap()
    hidT = nc.dram_tensor("hidT", [HT, 128, BS], F32R, kind="ExternalInput").ap()
    wet = nc.dram_tensor("wet", [FT // 2, 128, 2 * D], BF16, kind="ExternalInput").ap()
    w2T = nc.dram_tensor("w2T", [F, C2], BF16, kind="ExternalInput").ap()
    wot = nc.dram_tensor("wot", [CT, 128, H], F32R, kind="ExternalInput").ap()
    b1 = nc.dram_tensor("b1", [F], F32, kind="ExternalInput").ap()
    b2 = nc.dram_tensor("b2", [C2], BF16, kind="ExternalInput").ap()
    bo = nc.dram_tensor("bo", [H], F32R, kind="ExternalInput").ap()
    onesb = nc.dram_tensor("onesb", [128], BF16, kind="ExternalInput").ap()
    onesr = nc.dram_tensor("onesr", [128], F32R, kind="ExternalInput").ap()
    out = nc.dram_tensor("out", [BS, C2], F32, kind="ExternalOutput").ap()
    nh = nc.dram_tensor("nh", [BS, H], F32, kind="ExternalOutput").ap()

    def ring(i):
        return nc.sync if i % 2 == 0 else nc.gpsimd

    with tile.TileContext(nc) as tc:
        with (
            tc.tile_pool(name="consts", bufs=1) as consts,
            tc.tile_pool(name="psum", bufs=8, space="PSUM") as psum,
            tc.tile_pool(name="mid", bufs=1) as mid,
        ):
            # biases / ones
            b1_sb = consts.tile([128, FT], F32, tag="b1")
            nc.scalar.dma_start(out=b1_sb[:], in_=b1.rearrange("(t p) -> p t", p=128))
            b2_sb = consts.tile([1, C2], BF16, tag="b2")
            nc.scalar.dma_start(out=b2_sb[:], in_=b2[None, :])
            bo_sb = consts.tile([1, H], F32R, tag="bo")
            nc.scalar.dma_start(out=bo_sb[:], in_=bo[None, :])
            ones_b = consts.tile([1, 128], BF16, tag="onesb")
            nc.scalar.dma_start(out=ones_b[:], in_=onesb[None, :])
            ones_r = consts.tile([1, 128], F32R, tag="onesr")
            nc.scalar.dma_start(out=ones_r[:], in_=onesr[None, :])

            x_sb = consts.tile([128, DT * BS], BF16, tag="x")
            for t in range(DT):
                ring(t).dma_start(out=x_sb[:, t * BS : (t + 1) * BS], in_=xTb[t])

            h1_sb = mid.tile([128, FT * BS], BF16, tag="h1")

            def xs(t):
                return x_sb[:, t * BS : (t + 1) * BS]

            with (
                tc.tile_pool(name="xp2", bufs=1) as xp2,
                tc.tile_pool(name="wop", bufs=4) as wop,
                tc.tile_pool(name="w2p", bufs=6) as w2p,
                tc.tile_pool(name="nhp", bufs=1) as nhp,
            ):
                # ---- phase C: h1T[f, b] = relu(W_eff @ x + b_eff) ----
                x2_sb = xp2.tile([128, DT * BS], F32R, tag="x2")
                hid_sb = xp2.tile([128, HT * BS], F32R, tag="hid")
                with tc.tile_pool(name="w1p", bufs=4) as w1p:
                    for fc in range(FT // 2):
                        chunk = w1p.tile([128, 2 * D], BF16, tag="w1")
                        ring(fc).dma_start(out=chunk[:], in_=wet[fc])
                        for fi in range(2):
                            f = 2 * fc + fi
                            ps = psum.tile([128, BS], F32, tag="ps")
                            for c in range(DT):
                                nc.tensor.matmul(
                                    ps[:],
                                    chunk[:, (fi * DT + c) * 128 : (fi * DT + c + 1) * 128],
                                    xs(c),
                                    start=(c == 0),
                                    stop=(c == DT - 1),
                                )
                            nc.scalar.activation(
                                h1_sb[:, f * BS : (f + 1) * BS],
                                ps[:],
                                AFT.Relu,
                                bias=b1_sb[:, f : f + 1],
                            )
                        # interleave phase-E input prefetch into C's stream
                        if 2 <= fc < 2 + DT:
                            t = fc - 2
                            nc.scalar.dma_start(
                                out=x2_sb[:, t * BS : (t + 1) * BS], in_=xT[t]
                            )
                        if 8 <= fc < 8 + HT:
                            t = fc - 8
                            nc.scalar.dma_start(
                                out=hid_sb[:, t * BS : (t + 1) * BS], in_=hidT[t]
                            )

                # ---- phase E: nh[b, h] = cat(x, hid) @ w_out.T + b_out ----
                def comb(k):
                    if k < DT:
                        return x2_sb[:, k * BS : (k + 1) * BS]
                    return hid_sb[:, (k - DT) * BS : (k - DT + 1) * BS]

                nh_sb = nhp.tile([128, MT * H], F32, tag="nh")
                ps_e = []
                for j in range(8):
                    n2 = j % 2
                    ps = psum.tile([128, BS], F32, tag="ps")
                    nc.tensor.matmul(
                        ps[:],
                        ones_r[:],
                        bo_sb[:, n2 * 512 : (n2 + 1) * 512],
                        start=True,
                        stop=False,
                    )
                    ps_e.append(ps)
                for k in range(CT):
                    chunk = wop.tile([128, H], F32R, tag="wo")
                    ring(k).dma_start(out=chunk[:], in_=wot[k])
                    for j in range(8):
                        m, n2 = j // 2, j % 2
                        nc.tensor.matmul(
                            ps_e[j][:],
                            comb(k)[:, m * 128 : (m + 1) * 128],
                            chunk[:, n2 * 512 : (n2 + 1) * 512],
                            start=False,
                            stop=(k == CT - 1),
                        )
                for j in range(8):
                    m, n2 = j // 2, j % 2
                    dst = nh_sb[:, m * H + n2 * 512 : m * H + (n2 + 1) * 512]
                    if j % 2 == 0:
                        nc.vector.tensor_copy(dst, ps_e[j][:])
                    else:
                        nc.scalar.activation(dst, ps_e[j][:], AFT.Copy)
                for m in range(MT):
                    nc.sync.dma_start(
                        out=nh[m * 128 : (m + 1) * 128, :],
                        in_=nh_sb[:, m * H : (m + 1) * H],
                    )

                # ---- phase D: ffn_out = h1T.T @ w2T + b2; log_softmax ----
                with (
                    tc.tile_pool(name="fop", bufs=4) as fop,
                    tc.tile_pool(name="smp", bufs=2) as smp,
                ):
                    fo = [
                        fop.tile([128, C2], F32, tag="fo", name=f"fo{m}")
                        for m in range(MT)
                    ]
                    nmx0 = smp.tile([128, MT], F32, tag="nmx0")
                    s0s = smp.tile([128, MT], F32, tag="s0s")
                    s1s = smp.tile([128, MT], F32, tag="s1s")
                    dl0 = smp.tile([128, MT], F32, tag="dl0")
                    sms = smp.tile([128, MT], F32, tag="sms")
                    lns = smp.tile([128, MT], F32, tag="lns")
                    for n2 in range(2):
                        ps_d = []
                        for j in range(8):
                            nn = j % 2
                            ps = psum.tile([128, BS], F32, tag="ps")
                            nc.tensor.matmul(
                                ps[:],
                                ones_b[:],
                                b2_sb[
                                    :,
                                    n2 * 1024 + nn * 512 : n2 * 1024 + (nn + 1) * 512,
                                ],
                                start=True,
                                stop=False,
                            )
                            ps_d.append(ps)
                        for k in range(FT):
                            chunk = w2p.tile([128, 1024], BF16, tag="w2")
                            ring(k).dma_start(
                                out=chunk[:],
                                in_=w2T[
                                    k * 128 : (k + 1) * 128,
                                    n2 * 1024 : (n2 + 1) * 1024,
                                ],
                            )
                            for j in range(8):
                                m, nn = j // 2, j % 2
                                nc.tensor.matmul(
                                    ps_d[j][:],
                                    h1_sb[
                                        :, k * BS + m * 128 : k * BS + (m + 1) * 128
                                    ],
                                    chunk[:, nn * 512 : (nn + 1) * 512],
                                    start=False,
                                    stop=(k == FT - 1),
                                )
                        for j in range(8):
                            m, nn = j // 2, j % 2
                            dst = fo[m][
                                :, n2 * 1024 + nn * 512 : n2 * 1024 + (nn + 1) * 512
                            ]
                            if j % 2 == 0:
                                nc.vector.tensor_copy(dst, ps_d[j][:])
                            else:
                                nc.scalar.activation(dst, ps_d[j][:], AFT.Copy)
                        if n2 == 0:
                            # first-half row max + exp-sum, hidden under the
                            # n2=1 MM phase
                            for m in range(MT):
                                nc.vector.reduce_max(
                                    nmx0[:, m : m + 1],
                                    fo[m][:, :1024],
                                    axis=AX.X,
                                    negate=True,
                                )
                            for m in range(MT):
                                ex = smp.tile([128, 1024], F32, tag="ex")
                                nc.scalar.activation(
                                    ex[:],
                                    fo[m][:, :1024],
                                    AFT.Exp,
                                    bias=nmx0[:, m : m + 1],
                                    accum_out=s0s[:, m : m + 1],
                                )

                    # tail: exp-sum the second half with the SAME shift as the
                    # first half (values are O(5), so exp(x - m0) cannot
                    # overflow even if the true max is in the second half);
                    # then lse = ln(s0 + s1) - nmx0 exactly.
                    for m in range(MT):
                        ex = smp.tile([128, 1024], F32, tag="ex")
                        nc.scalar.activation(
                            ex[:],
                            fo[m][:, 1024:],
                            AFT.Exp,
                            bias=nmx0[:, m : m + 1],
                            accum_out=s1s[:, m : m + 1],
                        )
                    nc.vector.tensor_tensor(sms[:], s0s[:], s1s[:], op=ALU.add)
                    nc.scalar.activation(lns[:], sms[:], AFT.Ln)
                    # shift = nmx0 - lns, applied as out = fo + shift
                    nc.vector.tensor_tensor(dl0[:], nmx0[:], lns[:], op=ALU.subtract)
                    for m in range(MT):
                        if m != 1:
                            nc.vector.tensor_scalar(
                                out=fo[m][:],
                                in0=fo[m][:],
                                scalar1=dl0[:, m : m + 1],
                                scalar2=None,
                                op0=ALU.add,
                            )
                        else:
                            nc.scalar.activation(
                                fo[m][:],
                                fo[m][:],
                                AFT.Identity,
                                bias=dl0[:, m : m + 1],
                            )
                        nc.sync.dma_start(
                            out=out[m * 128 : (m + 1) * 128, :], in_=fo[m][:]
                        )

    nc.compile()
    return nc


_NC = None


def _get_nc():
    global _NC
    if _NC is None:
        _NC = _build_nc()
    return _NC


def _prep_host(inputs, hidden, w_k, b_k, w_proj, b_proj, ffn_w1, ffn_b1,
               ffn_w2, ffn_b2, w_out, b_out):
    f32 = np.float32
    bf16 = ml_dtypes.bfloat16
    asf = lambda a: np.asarray(a, dtype=f32)

    w_k = asf(w_k)
    w_proj = asf(w_proj)
    ffn_w1 = asf(ffn_w1)
    # fold the (linear) attention branch into the first FFN layer
    W1x, W1a = ffn_w1[:, :D], ffn_w1[:, D:]
    M1 = w_proj @ w_k  # [A, D]
    W_eff = W1x + W1a @ M1  # [F, D]
    b_eff = asf(ffn_b1) + W1a @ (w_proj @ asf(b_k) + asf(b_proj))

    W_effT = np.ascontiguousarray(W_eff.T)  # [D, F]
    wet = (
        W_effT.reshape(DT, 128, FT // 2, 2, 128)
        .transpose(2, 1, 3, 0, 4)
        .reshape(FT // 2, 128, 2 * D)
        .astype(bf16)
    )
    w2T = np.ascontiguousarray(asf(ffn_w2).T).astype(bf16)  # [F, C2]
    wot = np.ascontiguousarray(asf(w_out).T).reshape(CT, 128, H)

    shared = {
        "wet": np.ascontiguousarray(wet), "w2T": w2T, "wot": wot,
        "b1": b_eff.astype(f32), "b2": asf(ffn_b2).astype(bf16),
        "bo": asf(b_out),
        "onesb": np.ones((128,), bf16), "onesr": np.ones((128,), f32),
    }
    x = asf(inputs)
    hd = asf(hidden)
    in_maps = []
    for c in range(NCORES):
        xs = np.ascontiguousarray(x[c * BS : (c + 1) * BS].T).reshape(DT, 128, BS)
        hs = np.ascontiguousarray(hd[c * BS : (c + 1) * BS].T).reshape(HT, 128, BS)
        in_maps.append({**shared, "xT": xs, "xTb": xs.astype(bf16), "hidT": hs})
    return in_maps


def kernel(inputs, hidden, w_k, b_k, w_q, b_q, mlp_w, w_proj, b_proj,
           ffn_w1, ffn_b1, ffn_w2, ffn_b2, w_out, b_out):
    # w_q / b_q / mlp_w only feed the attention score, and softmax over the
    # size-1 k_len axis makes the score identically 1.0 -> they are unused.
    in_maps = _prep_host(inputs, hidden, w_k, b_k, w_proj, b_proj,
                         ffn_w1, ffn_b1, ffn_w2, ffn_b2, w_out, b_out)
    res = run_bass_kernel_spmd(_get_nc(), in_maps, list(range(NCORES))).results
    out = np.concatenate([r["out"] for r in res], axis=0)
    nh = np.concatenate([r["nh"] for r in res], axis=0)
    return out, nh


# revision 17
# speedup vs baseline: 1.1734x; 1.1734x over previous
"""Trainium2 Bass kernel for nn_AttentionRNNCell (B=4096, D=H=A=1024, F=4096).

Math (after structural simplification of the reference):
  1) softmax over the size-1 k_len axis is identically 1.0, so
     attn = kx = inputs @ w_k.T + b_k, and qx / mlp_w / tanh are dead code.
  2) attn_out is then a LINEAR map of inputs, so the first FFN layer folds
     over it exactly (host-side fp32 precompute):
        ffn_w1 = [W1x | W1a]   (split along the input dim: D then A)
        W_eff  = W1x + W1a @ (w_proj @ w_k)            [F, D]
        b_eff  = ffn_b1 + W1a @ (w_proj @ b_k + b_proj)
  Device computation per batch row:
    h1      = relu(x @ W_eff.T + b_eff)          [B, F]   (K=1024)
    ffn_out = h1 @ ffn_w2.T + ffn_b2             [B, D+A] (K=4096)
    new_h   = cat(x, hidden) @ w_out.T + b_out   [B, H]   (K=2048)
    output  = log_softmax(ffn_out, axis=-1)
  returns (output, new_h)

Sharding: data-parallel over B across 8 cores (512 rows/core), weights
replicated.  The log_softmax branch (C/D) runs bf16 matmuls with fp32 PSUM
accumulation -- log_softmax normalizes away most quantization noise.  The
new_hidden GEMM (E) runs fp32r (FP22 multiply, fp32 accumulate) since that
output has no normalizer.  Activations are feature-major ([feat, batch]) in
C; D and E put batch on partitions (activations stationary) so log_softmax
reduces along the free axis.  Biases fuse into the PSUM->SBUF activation
copy (C) or enter PSUM via K=1 ones-matmuls (D/E).  Weight streams alternate
between the HWDGE (sync) and SWDGE (gpsimd) DMA rings.
"""

import numpy as np
import ml_dtypes

import concourse.bass as bass
import concourse.mybir as mybir
import concourse.tile as tile
from concourse import bacc
from concourse.bass_utils import run_bass_kernel_spmd

F32 = mybir.dt.float32
F32R = mybir.dt.float32r
BF16 = mybir.dt.bfloat16
AFT = mybir.ActivationFunctionType
ALU = mybir.AluOpType
AX = mybir.AxisListType

NCORES = 8
B, D, H, A, F = 4096, 1024, 1024, 1024, 4096
C2 = D + A  # 2048, ffn output width and k-dim of w_out
BS = B // NCORES  # 512 batch rows per core

DT = D // 128  # 8   d-tiles (K of the fused first GEMM)
HT = H // 128  # 8   h-tiles
CT = C2 // 128  # 16  c-tiles (K of the w_out GEMM)
FT = F // 128  # 32  f-tiles (K of the second FFN GEMM)
MT = BS // 128  # 4   batch tiles per core


def _build_nc():
    nc = bacc.Bacc("TRN2", target_bir_lowering=False, debug=False)

    # ---- DRAM I/O (per-core shapes; host pre-tiles everything) ----
    xTb = nc.dram_tensor("xTb", [DT, 128, BS], BF16, kind="ExternalInput").ap()
    xT = nc.dram_tensor("xT", [DT, 128, BS], F32R, kind="ExternalInput").ap()
    hidT = nc.dram_tensor("hidT", [HT, 128, BS], F32R, kind="ExternalInput").ap()
    wet = nc.dram_tensor("wet", [FT // 2, 128, 2 * D], BF16, kind="ExternalInput").ap()
    w2T = nc.dram_tensor("w2T", [F, C2], BF16, kind="ExternalInput").ap()
    wot = nc.dram_tensor("wot", [CT, 128, H], F32R, kind="ExternalInput").ap()
    b1 = nc.dram_tensor("b1", [F], F32, kind="ExternalInput").ap()
    b2 = nc.dram_tensor("b2", [C2], BF16, kind="ExternalInput").ap()
    bo = nc.dram_tensor("bo", [H], F32R, kind="ExternalInput").ap()
    onesb = nc.dram_tensor("onesb", [128], BF16, kind="ExternalInput").ap()
    onesr = nc.dram_tensor("onesr", [128], F32R, kind="ExternalInput").ap()
    out = nc.dram_tensor("out", [BS, C2], F32, kind="ExternalOutput").ap()
    nh = nc.dram_tensor("nh", [BS, H], F32, kind="ExternalOutput").ap()

    def ring(i):
        return nc.sync if i % 2 == 0 else nc.gpsimd

    with tile.TileContext(nc) as tc:
        with (
            tc.tile_pool(name="consts", bufs=1) as consts,
            tc.tile_pool(name="psum", bufs=8, space="PSUM") as psum,
            tc.tile_pool(name="mid", bufs=1) as mid,
        ):
            # biases / ones
            b1_sb = consts.tile([128, FT], F32, tag="b1")
            nc.scalar.dma_start(out=b1_sb[:], in_=b1.rearrange("(t p) -> p t", p=128))
            b2_sb = consts.tile([1, C2], BF16, tag="b2")
            nc.scalar.dma_start(out=b2_sb[:], in_=b2[None, :])
            bo_sb = consts.tile([1, H], F32R, tag="bo")
            nc.scalar.dma_start(out=bo_sb[:], in_=bo[None, :])
            ones_b = consts.tile([1, 128], BF16, tag="onesb")
            nc.scalar.dma_start(out=ones_b[:], in_=onesb[None, :])
            ones_r = consts.tile([1, 128], F32R, tag="onesr")
            nc.scalar.dma_start(out=ones_r[:], in_=onesr[None, :])

            x_sb = consts.tile([128, DT * BS], BF16, tag="x")
            for t in range(DT):
                ring(t).dma_start(out=x_sb[:, t * BS : (t + 1) * BS], in_=xTb[t])

            h1_sb = mid.tile([128, FT * BS], BF16, tag="h1")

            def xs(t):
                return x_sb[:, t * BS : (t + 1) * BS]

            with (
                tc.tile_pool(name="xp2", bufs=1) as xp2,
                tc.tile_pool(name="wop", bufs=4) as wop,
                tc.tile_pool(name="w2p", bufs=6) as w2p,
                tc.tile_pool(name="nhp", bufs=1) as nhp,
            ):
                # ---- phase C: h1T[f, b] = relu(W_eff @ x + b_eff) ----
                x2_sb = xp2.tile([128, DT * BS], F32R, tag="x2")
                hid_sb = xp2.tile([128, HT * BS], F32R, tag="hid")
                with tc.tile_pool(name="w1p", bufs=5) as w1p:
                    for fc in range(FT // 2):
                        chunk = w1p.tile([128, 2 * D], BF16, tag="w1")
                        ring(fc).dma_start(out=chunk[:], in_=wet[fc])
                        for fi in range(2):
                            f = 2 * fc + fi
                            ps = psum.tile([128, BS], F32, tag="ps")
                            for c in range(DT):
                                nc.tensor.matmul(
                                    ps[:],
                                    chunk[:, (fi * DT + c) * 128 : (fi * DT + c + 1) * 128],
                                    xs(c),
                                    start=(c == 0),
                                    stop=(c == DT - 1),
                                )
                            nc.scalar.activation(
                                h1_sb[:, f * BS : (f + 1) * BS],
                                ps[:],
                                AFT.Relu,
                                bias=b1_sb[:, f : f + 1],
                            )
                        # interleave phase-E input prefetch into C's stream
                        if 6 <= fc < 6 + DT:
                            t = fc - 6
                            ring(t).dma_start(
                                out=x2_sb[:, t * BS : (t + 1) * BS], in_=xT[t]
                            )
                        if 8 <= fc < 8 + HT:
                            t = fc - 8
                            ring(t + 1).dma_start(
                                out=hid_sb[:, t * BS : (t + 1) * BS], in_=hidT[t]
                            )

                # ---- phase E: nh[b, h] = cat(x, hid) @ w_out.T + b_out ----
                def comb(k):
                    if k < DT:
                        return x2_sb[:, k * BS : (k + 1) * BS]
                    return hid_sb[:, (k - DT) * BS : (k - DT + 1) * BS]

                nh_sb = nhp.tile([128, MT * H], F32, tag="nh")
                ps_e = []
                for j in range(8):
                    n2 = j % 2
                    ps = psum.tile([128, BS], F32, tag="ps")
                    nc.tensor.matmul(
                        ps[:],
                        ones_r[:],
                        bo_sb[:, n2 * 512 : (n2 + 1) * 512],
                        start=True,
                        stop=False,
                    )
                    ps_e.append(ps)
                for k in range(CT):
                    chunk = wop.tile([128, H], F32R, tag="wo")
                    ring(k).dma_start(out=chunk[:], in_=wot[k])
                    for j in range(8):
                        m, n2 = j // 2, j % 2
                        nc.tensor.matmul(
                            ps_e[j][:],
                            comb(k)[:, m * 128 : (m + 1) * 128],
                            chunk[:, n2 * 512 : (n2 + 1) * 512],
                            start=False,
                            stop=(k == CT - 1),
                        )
                for j in range(8):
                    m, n2 = j // 2, j % 2
                    dst = nh_sb[:, m * H + n2 * 512 : m * H + (n2 + 1) * 512]
                    if j % 2 == 0:
                        nc.vector.tensor_copy(dst, ps_e[j][:])
                    else:
                        nc.scalar.activation(dst, ps_e[j][:], AFT.Copy)
                for m in range(MT):
                    nc.sync.dma_start(
                        out=nh[m * 128 : (m + 1) * 128, :],
                        in_=nh_sb[:, m * H : (m + 1) * H],
                    )

                # ---- phase D: ffn_out = h1T.T @ w2T + b2; log_softmax ----
                with (
                    tc.tile_pool(name="fop", bufs=4) as fop,
                    tc.tile_pool(name="smp", bufs=2) as smp,
                ):
                    fo = [
                        fop.tile([128, C2], F32, tag="fo", name=f"fo{m}")
                        for m in range(MT)
                    ]
                    nmx0 = smp.tile([128, MT], F32, tag="nmx0")
                    s0s = smp.tile([128, MT], F32, tag="s0s")
                    s1s = smp.tile([128, MT], F32, tag="s1s")
                    dl0 = smp.tile([128, MT], F32, tag="dl0")
                    sms = smp.tile([128, MT], F32, tag="sms")
                    lns = smp.tile([128, MT], F32, tag="lns")
                    for n2 in range(2):
                        ps_d = []
                        for j in range(8):
                            nn = j % 2
                            ps = psum.tile([128, BS], F32, tag="ps")
                            nc.tensor.matmul(
                                ps[:],
                                ones_b[:],
                                b2_sb[
                                    :,
                                    n2 * 1024 + nn * 512 : n2 * 1024 + (nn + 1) * 512,
                                ],
                                start=True,
                                stop=False,
                            )
                            ps_d.append(ps)
                        for k in range(FT):
                            chunk = w2p.tile([128, 1024], BF16, tag="w2")
                            ring(k).dma_start(
                                out=chunk[:],
                                in_=w2T[
                                    k * 128 : (k + 1) * 128,
                                    n2 * 1024 : (n2 + 1) * 1024,
                                ],
                            )
                            for j in range(8):
                                m, nn = j // 2, j % 2
                                nc.tensor.matmul(
                                    ps_d[j][:],
                                    h1_sb[
                                        :, k * BS + m * 128 : k * BS + (m + 1) * 128
                                    ],
                                    chunk[:, nn * 512 : (nn + 1) * 512],
                                    start=False,
                                    stop=(k == FT - 1),
                                )
                        for j in range(8):
                            m, nn = j // 2, j % 2
                            dst = fo[m][
                                :, n2 * 1024 + nn * 512 : n2 * 1024 + (nn + 1) * 512
                            ]
                            if j % 2 == 0:
                                nc.vector.tensor_copy(dst, ps_d[j][:])
                            else:
                                nc.scalar.activation(dst, ps_d[j][:], AFT.Copy)
                        if n2 == 0:
                            # first-half row max + exp-sum, hidden under the
                            # n2=1 MM phase
                            for m in range(MT):
                                nc.vector.reduce_max(
                                    nmx0[:, m : m + 1],
                                    fo[m][:, :1024],
                                    axis=AX.X,
                                    negate=True,
                                )
                            for m in range(MT):
                                ex = smp.tile([128, 1024], F32, tag="ex")
                                nc.scalar.activation(
                                    ex[:],
                                    fo[m][:, :1024],
                                    AFT.Exp,
                                    bias=nmx0[:, m : m + 1],
                                    accum_out=s0s[:, m : m + 1],
                                )

                    # tail: exp-sum the second half with the SAME shift as the
                    # first half (values are O(5), so exp(x - m0) cannot
                    # overflow even if the true max is in the second half);
                    # then lse = ln(s0 + s1) - nmx0 exactly.
                    for m in range(MT):
                        ex = smp.tile([128, 1024], F32, tag="ex")
                        nc.scalar.activation(
                            ex[:],
                            fo[m][:, 1024:],
                            AFT.Exp,
                            bias=nmx0[:, m : m + 1],
                            accum_out=s1s[:, m : m + 1],
                        )
                    nc.vector.tensor_tensor(sms[:], s0s[:], s1s[:], op=ALU.add)
                    nc.scalar.activation(lns[:], sms[:], AFT.Ln)
                    # shift = nmx0 - lns, applied as out = fo + shift
                    nc.vector.tensor_tensor(dl0[:], nmx0[:], lns[:], op=ALU.subtract)
                    for m in range(MT):
                        if m != 1:
                            nc.vector.tensor_scalar(
                                out=fo[m][:],
                                in0=fo[m][:],
                                scalar1=dl0[:, m : m + 1],
                                scalar2=None,
                                op0=ALU.add,
                            )
                        else:
                            nc.scalar.activation(
                                fo[m][:],
                                fo[m][:],
                                AFT.Identity,
                                bias=dl0[:, m : m + 1],
                            )
                        nc.sync.dma_start(
                            out=out[m * 128 : (m + 1) * 128, :], in_=fo[m][:]
                        )

    nc.compile()
    return nc


_NC = None


def _get_nc():
    global _NC
    if _NC is None:
        _NC = _build_nc()
    return _NC


def _prep_host(inputs, hidden, w_k, b_k, w_proj, b_proj, ffn_w1, ffn_b1,
               ffn_w2, ffn_b2, w_out, b_out):
    f32 = np.float32
    bf16 = ml_dtypes.bfloat16
    asf = lambda a: np.asarray(a, dtype=f32)

    w_k = asf(w_k)
    w_proj = asf(w_proj)
    ffn_w1 = asf(ffn_w1)
    # fold the (linear) attention branch into the first FFN layer
    W1x, W1a = ffn_w1[:, :D], ffn_w1[:, D:]
    M1 = w_proj @ w_k  # [A, D]
    W_eff = W1x + W1a @ M1  # [F, D]
    b_eff = asf(ffn_b1) + W1a @ (w_proj @ asf(b_k) + asf(b_proj))

    W_effT = np.ascontiguousarray(W_eff.T)  # [D, F]
    wet = (
        W_effT.reshape(DT, 128, FT // 2, 2, 128)
        .transpose(2, 1, 3, 0, 4)
        .reshape(FT // 2, 128, 2 * D)
        .astype(bf16)
    )
    w2T = np.ascontiguousarray(asf(ffn_w2).T).astype(bf16)  # [F, C2]
    wot = np.ascontiguousarray(asf(w_out).T).reshape(CT, 128, H)

    shared = {
        "wet": np.ascontiguousarray(wet), "w2T": w2T, "wot": wot,
        "b1": b_eff.astype(f32), "b2": asf(ffn_b2).astype(bf16),
        "bo": asf(b_out),
        "onesb": np.ones((128,), bf16), "onesr": np.ones((128,), f32),
    }
    x = asf(inputs)
    hd = asf(hidden)
    in_maps = []
    for c in range(NCORES):
        xs = np.ascontiguousarray(x[c * BS : (c + 1) * BS].T).reshape(DT, 128, BS)
        hs = np.ascontiguousarray(hd[c * BS : (c + 1) * BS].T).reshape(HT, 128, BS)
        in_maps.append({**shared, "xT": xs, "xTb": xs.astype(bf16), "hidT": hs})
    return in_maps


def kernel(inputs, hidden, w_k, b_k, w_q, b_q, mlp_w, w_proj, b_proj,
           ffn_w1, ffn_b1, ffn_w2, ffn_b2, w_out, b_out):
    # w_q / b_q / mlp_w only feed the attention score, and softmax over the
    # size-1 k_len axis makes the score identically 1.0 -> they are unused.
    in_maps = _prep_host(inputs, hidden, w_k, b_k, w_proj, b_proj,
                         ffn_w1, ffn_b1, ffn_w2, ffn_b2, w_out, b_out)
    res = run_bass_kernel_spmd(_get_nc(), in_maps, list(range(NCORES))).results
    out = np.concatenate([r["out"] for r in res], axis=0)
    nh = np.concatenate([r["nh"] for r in res], axis=0)
    return out, nh
